# revision 6
# baseline (speedup 1.0000x reference)
"""GRU kernel for Trainium2 (8 NeuronCores, SPMD).

Problem: B=64, T=512, IN=256, H=1024, OUT=10
  gate_x_all = x @ Wx.T + bx            [B,T,3H]
  scan over T:  gate_h = h @ Wh.T + bh
                r = sig(i_r + h_r); i = sig(i_i + h_i)
                n = tanh(i_n + r * h_n)
                h = n + i * (h - n)
  out = h_last @ Wfc.T + bfc            [B,OUT]

v0 strategy: data-parallel over batch (8 rows per core), weights replicated.
All layout preparation (transposes, batch slicing) is done host-side in
kernel(); the device program uses feature-major (K-on-partitions) operands
for matmuls directly.
"""

import os
import sys

import numpy as np

for p in ("/root/.axon_site", "/root/.axon_site/_ro/trn_rl_repo",
          "/root/.axon_site/_ro/pypackages", "/opt/trn_rl_repo"):
    if p not in sys.path:
        sys.path.append(p)

B, T, IN, H, OUT = 64, 512, 256, 1024, 10
H3 = 3 * H
NCORES = 8
BS = B // NCORES  # batch per core (8)

_cache = {}


def _build_program():
    import concourse.bass as bass
    import concourse.bacc as bacc
    import concourse.mybir as mybir
    from concourse.tile import TileContext

    fp32 = mybir.dt.float32
    AF = mybir.ActivationFunctionType

    nc = bacc.Bacc(None, target_bir_lowering=False)

    # ---- per-core inputs (feature-major layouts prepared on host) ----
    xsT_d = nc.dram_tensor("xsT", [T, 2, 128, BS], fp32, kind="ExternalInput")
    whT_d = nc.dram_tensor("whT", [8, 128, H3], fp32, kind="ExternalInput")
    wxT_d = nc.dram_tensor("wxT", [2, 128, H3], fp32, kind="ExternalInput")
    bias_x_d = nc.dram_tensor("bias_x", [1, H3], fp32, kind="ExternalInput")
    bias_hn_d = nc.dram_tensor("bias_hn", [1, H], fp32, kind="ExternalInput")
    wfcT_d = nc.dram_tensor("wfcT", [8, 128, OUT], fp32, kind="ExternalInput")
    bfc_d = nc.dram_tensor("bfc", [1, OUT], fp32, kind="ExternalInput")
    ident_d = nc.dram_tensor("ident", [128, 128], fp32, kind="ExternalInput")
    out_d = nc.dram_tensor("out", [BS, OUT], fp32, kind="ExternalOutput")

    t_steps = int(os.environ.get("KERNEL_T", str(T)))

    with TileContext(nc) as tc:
        with (
            tc.tile_pool(name="const", bufs=1) as cpool,
            tc.tile_pool(name="state", bufs=1) as spool,
            tc.tile_pool(name="xt", bufs=8) as xpool,
            tc.tile_pool(name="gates", bufs=2) as gpool,
            tc.tile_pool(name="pg", bufs=1, space="PSUM") as pgpool,
            tc.tile_pool(name="phn", bufs=1, space="PSUM") as phnpool,
        ):
            # constants -- loaded in per-tile chunks so no consumer has to
            # wait on a many-queue giant DMA.
            whT = cpool.tile([128, 8, H3], fp32)
            for k in range(8):
                for c in range(6):
                    cs = slice(c * 512, (c + 1) * 512)
                    nc.sync.dma_start(whT[:, k, cs], whT_d[k, :, cs])
            wxT = cpool.tile([128, 2, H3], fp32)
            for k in range(2):
                for c in range(6):
                    cs = slice(c * 512, (c + 1) * 512)
                    nc.sync.dma_start(wxT[:, k, cs], wxT_d[k, :, cs])
            bias_x = cpool.tile([1, H3], fp32)
            nc.sync.dma_start(bias_x[:], bias_x_d[:])
            bias_hn = cpool.tile([1, H], fp32)
            nc.sync.dma_start(bias_hn[:], bias_hn_d[:])
            wfcT = cpool.tile([128, 8, OUT], fp32)
            for k in range(8):
                nc.sync.dma_start(wfcT[:, k, :], wfcT_d[k])
            bfc = cpool.tile([1, OUT], fp32)
            nc.sync.dma_start(bfc[:], bfc_d[:])
            ident = cpool.tile([128, 128], fp32)
            nc.sync.dma_start(ident[:], ident_d[:])
            ones = cpool.tile([1, BS], fp32)
            nc.gpsimd.memset(ones[:], 1.0)

            # state
            hT = spool.tile([128, 8, BS], fp32)     # feature-major h (K-blocks)
            nc.gpsimd.memset(hT[:], 0.0)
            h_bm = spool.tile([BS, H], fp32)        # batch-major h
            nc.gpsimd.memset(h_bm[:], 0.0)

            def step(i):
                xt = xpool.tile([128, 2, BS], fp32, tag="xt")
                for k in range(2):
                    nc.sync.dma_start(xt[:, k, :], xsT_d[i, k])

                pg = pgpool.tile([BS, H3], fp32, tag="pg")
                phn = phnpool.tile([BS, H], fp32, tag="phn")

                # x contribution: all three gates
                for c in range(6):
                    cs = slice(c * 512, (c + 1) * 512)
                    for k in range(2):
                        nc.tensor.matmul(pg[:, cs], xt[:, k, :], wxT[:, k, cs],
                                         start=(k == 0), stop=False)
                # h contribution: r,i gates accumulate into pg
                for c in range(4):
                    cs = slice(c * 512, (c + 1) * 512)
                    for k in range(8):
                        nc.tensor.matmul(pg[:, cs], hT[:, k, :], whT[:, k, cs],
                                         start=False, stop=False)
                # h contribution: n gate separate
                for c in range(2):
                    cs = slice(c * 512, (c + 1) * 512)
                    for k in range(8):
                        nc.tensor.matmul(phn[:, cs], hT[:, k, :],
                                         whT[:, k, 2048 + c * 512:2048 + (c + 1) * 512],
                                         start=(k == 0), stop=False)
                # biases via ones-row matmuls
                for c in range(6):
                    cs = slice(c * 512, (c + 1) * 512)
                    nc.tensor.matmul(pg[:, cs], ones[:], bias_x[:, cs],
                                     start=False, stop=True)
                for c in range(2):
                    cs = slice(c * 512, (c + 1) * 512)
                    nc.tensor.matmul(phn[:, cs], ones[:], bias_hn[:, cs],
                                     start=False, stop=True)

                # gates
                r_sb = gpool.tile([BS, H], fp32, tag="r")
                nc.scalar.activation(r_sb[:], pg[:, 0:H], AF.Sigmoid)
                i_sb = gpool.tile([BS, H], fp32, tag="i")
                nc.scalar.activation(i_sb[:], pg[:, H:2 * H], AF.Sigmoid)
                t1 = gpool.tile([BS, H], fp32, tag="t1")
                nc.vector.tensor_mul(t1[:], r_sb[:], phn[:])
                t2 = gpool.tile([BS, H], fp32, tag="t2")
                nc.vector.tensor_add(t2[:], t1[:], pg[:, 2 * H:3 * H])
                ng = gpool.tile([BS, H], fp32, tag="ng")
                nc.scalar.activation(ng[:], t2[:], AF.Tanh)
                d = gpool.tile([BS, H], fp32, tag="d")
                nc.vector.tensor_sub(d[:], h_bm[:], ng[:])
                u = gpool.tile([BS, H], fp32, tag="u")
                nc.vector.tensor_mul(u[:], i_sb[:], d[:])
                nc.vector.tensor_add(h_bm[:], ng[:], u[:])

                # transpose h_bm -> hT (feature-major) for next step's matmul
                for k in range(8):
                    pt = phnpool.tile([128, BS], fp32, tag="phn")
                    nc.tensor.transpose(pt[:], h_bm[:, k * 128:(k + 1) * 128],
                                        ident[:BS, :BS])
                    nc.vector.tensor_copy(hT[:, k, :], pt[:])

            for i in range(t_steps):
                step(i)

            # final FC
            po = phnpool.tile([BS, OUT], fp32, tag="phn")
            for k in range(8):
                nc.tensor.matmul(po[:], hT[:, k, :], wfcT[:, k, :],
                                 start=(k == 0), stop=False)
            nc.tensor.matmul(po[:], ones[:], bfc[:], start=False, stop=True)
            out_sb = gpool.tile([BS, OUT], fp32, tag="r")
            nc.vector.tensor_copy(out_sb[:], po[:])
            nc.sync.dma_start(out_d[:], out_sb[:])

    nc.compile()
    return nc


def _prep_inputs(x, Wx, bx, Wh, bh, Wfc, bfc):
    """Host-side layout prep -> list of per-core input dicts."""
    x = np.asarray(x, np.float32)
    Wx = np.asarray(Wx, np.float32)
    bx = np.asarray(bx, np.float32)
    Wh = np.asarray(Wh, np.float32)
    bh = np.asarray(bh, np.float32)
    Wfc = np.asarray(Wfc, np.float32)
    bfc = np.asarray(bfc, np.float32)

    whT = np.ascontiguousarray(Wh.T).reshape(8, 128, H3)
    wxT = np.ascontiguousarray(Wx.T).reshape(2, 128, H3)
    bias_x = (bx + np.concatenate([bh[:2 * H], np.zeros(H, np.float32)])).reshape(1, H3)
    bias_hn = bh[2 * H:].reshape(1, H)
    wfcT = np.ascontiguousarray(Wfc.T).reshape(8, 128, OUT)
    bfc2 = bfc.reshape(1, OUT)
    ident = np.eye(128, dtype=np.float32)

    in_maps = []
    for c in range(NCORES):
        xs = x[c * BS:(c + 1) * BS]              # [BS, T, IN]
        xsT = np.ascontiguousarray(xs.transpose(1, 2, 0))  # [T, IN, BS]
        in_maps.append({
            "xsT": xsT.reshape(T, 2, 128, BS),
            "whT": whT, "wxT": wxT,
            "bias_x": bias_x, "bias_hn": bias_hn,
            "wfcT": wfcT, "bfc": bfc2, "ident": ident,
        })
    return in_maps


def kernel(x, Wx, bx, Wh, bh, Wfc, bfc):
    from concourse.bass_utils import run_bass_kernel_spmd

    if "nc" not in _cache:
        _cache["nc"] = _build_program()
    nc = _cache["nc"]

    in_maps = _prep_inputs(x, Wx, bx, Wh, bh, Wfc, bfc)
    res = run_bass_kernel_spmd(nc, in_maps, list(range(NCORES)))
    out = np.concatenate([res.results[c]["out"] for c in range(NCORES)], axis=0)
    return out.astype(np.float32)


if __name__ == "__main__":
    rng = np.random.default_rng(0)
    std = 1.0 / np.sqrt(H)
    inputs = {
        "x": rng.standard_normal((B, T, IN), dtype=np.float32),
        "Wx": rng.uniform(-std, std, (H3, IN)).astype(np.float32),
        "bx": rng.uniform(-std, std, (H3,)).astype(np.float32),
        "Wh": rng.uniform(-std, std, (H3, H)).astype(np.float32),
        "bh": rng.uniform(-std, std, (H3,)).astype(np.float32),
        "Wfc": rng.uniform(-std, std, (OUT, H)).astype(np.float32),
        "bfc": rng.uniform(-std, std, (OUT,)).astype(np.float32),
    }
    out = kernel(**inputs)
    print("out", out.shape, out.dtype, out[:2])


# revision 15
# speedup vs baseline: 4.2010x; 4.2010x over previous
"""GRU kernel for Trainium2 (8 NeuronCores, SPMD).

Problem: B=64, T=512, IN=256, H=1024, OUT=10
  gate_x_all = x @ Wx.T + bx            [B,T,3H]
  scan over T:  gate_h = h @ Wh.T + bh
                r = sig(i_r + h_r); i = sig(i_i + h_i)
                n = tanh(i_n + r * h_n)
                h = n + i * (h - n)
  out = h_last @ Wfc.T + bfc            [B,OUT]

Strategy (v2): data-parallel over batch (8 rows/core), weights replicated.
 - bf16 operands, fp32 PSUM accumulation and fp32 gate math.
 - Phase 1: gate_x precomputed for all T in one large weight-streaming GEMM
   (output grouped by hidden-quarter, biases folded in via a ones-row).
 - Phase 2: the T-step scan. gate_h computed with 4x column-tiled matmuls
   (hidden quarter j -> PE column group j, PSUM partitions 32j..32j+8),
   so four weight streams flow through the PE concurrently.
   gate_x is re-injected into PSUM via tiny identity-stationary matmuls.
   Gate elementwise work is split across ACT / DVE / GPSIMD.
 - h is kept feature-major (hT) for the next matmul via PE transposes.
"""

import os
import sys

import numpy as np

for p in ("/root/.axon_site", "/root/.axon_site/_ro/trn_rl_repo",
          "/root/.axon_site/_ro/pypackages", "/opt/trn_rl_repo"):
    if p not in sys.path:
        sys.path.append(p)

B, T, IN, H, OUT = 64, 512, 256, 1024, 10
H3 = 3 * H
NCORES = 8
BS = B // NCORES      # batch per core (8)
Q = H // 4            # hidden quarter (256)
GF = 3 * Q            # per-group gate columns (768): [r 256 | i 256 | n 256]

_cache = {}


def _build_program():
    import concourse.bass as bass
    import concourse.bacc as bacc
    import concourse.mybir as mybir
    from concourse.tile import TileContext

    f32 = mybir.dt.float32
    b16 = mybir.dt.bfloat16
    AF = mybir.ActivationFunctionType

    nc = bacc.Bacc(None, target_bir_lowering=False)

    # ---- per-core inputs ----
    # x slice, feature-major for the precompute stationaries: [2,128,(t,b)]
    xsT_d = nc.dram_tensor("xsT", [2, 128, T * BS], b16, kind="ExternalInput")
    # Wx^T with grouped column order (+ biases row): [2,128, 4*768]
    wxg_d = nc.dram_tensor("wxg", [2, 128, H3], b16, kind="ExternalInput")
    biasg_d = nc.dram_tensor("biasg", [1, H3], b16, kind="ExternalInput")
    # Wh^T grouped: bank0 (r,i) and bankn (n) streams
    whb0_d = nc.dram_tensor("whb0", [8, 128, 4, 2 * Q], b16, kind="ExternalInput")
    whbn_d = nc.dram_tensor("whbn", [8, 128, 4, Q], b16, kind="ExternalInput")
    bhn_d = nc.dram_tensor("bhn", [1, 4, Q], b16, kind="ExternalInput")
    wfcT_d = nc.dram_tensor("wfcT", [8, 128, OUT], b16, kind="ExternalInput")
    bfc_d = nc.dram_tensor("bfc", [1, OUT], f32, kind="ExternalInput")
    ident_d = nc.dram_tensor("identg", [128, 8], b16, kind="ExternalInput")
    identf_d = nc.dram_tensor("identfg", [128, 8], f32, kind="ExternalInput")
    out_d = nc.dram_tensor("out", [BS, OUT], f32, kind="ExternalOutput")
    debug = os.environ.get("KERNEL_DEBUG", "0") == "1"
    if debug:
        dbg_gx_d = nc.dram_tensor("dbg_gx", [128, GF], f32, kind="ExternalOutput")
        dbg_ri_d = nc.dram_tensor("dbg_ri", [128, 2 * Q], f32, kind="ExternalOutput")
        dbg_pbn_d = nc.dram_tensor("dbg_pbn", [128, Q], f32, kind="ExternalOutput")
        dbg_hnew_d = nc.dram_tensor("dbg_hnew", [128, Q], f32, kind="ExternalOutput")
        dbg_hT_d = nc.dram_tensor("dbg_hT", [128, 8, BS], f32, kind="ExternalOutput")
        dbg_gxd_d = nc.dram_tensor("dbg_gxd", [BS, H3], b16, kind="ExternalOutput")

    # gate_x for all T, grouped: [m=T/16, p=(16t x 8b), 4*GF]
    gx_d = nc.dram_tensor("gx_all", [T // 16, 128, H3], b16)

    t_steps = int(os.environ.get("KERNEL_T", str(T)))

    with TileContext(nc) as tc:
        # ---------- constants ----------
        with tc.tile_pool(name="const", bufs=1) as cpool:
            wh0 = cpool.tile([128, 8, 4, 2 * Q], b16)
            for k in range(8):
                for j in range(4):
                    nc.sync.dma_start(wh0[:, k, j, :], whb0_d[k, :, j, :])
            whn = cpool.tile([128, 8, 4, Q], b16)
            for k in range(8):
                for j in range(4):
                    nc.sync.dma_start(whn[:, k, j, :], whbn_d[k, :, j, :])
            bhn = cpool.tile([1, 4, Q], b16)
            nc.sync.dma_start(bhn[:], bhn_d[:])
            wfcT = cpool.tile([128, 8, OUT], b16)
            for k in range(8):
                nc.sync.dma_start(wfcT[:, k, :], wfcT_d[k])
            bfc = cpool.tile([1, OUT], f32)
            nc.sync.dma_start(bfc[:], bfc_d[:])
            ones_b = cpool.tile([1, 128], b16)
            nc.gpsimd.memset(ones_b[:], 1.0)
            ones_f = cpool.tile([1, BS], f32)
            nc.gpsimd.memset(ones_f[:], 1.0)
            identg = cpool.tile([128, 8], b16)
            nc.sync.dma_start(identg[:], ident_d[:])
            identfg = cpool.tile([128, 8], f32)
            nc.sync.dma_start(identfg[:], identf_d[:])

            # ---------- phase 1: gate_x precompute ----------
            with (
                tc.tile_pool(name="px", bufs=2) as pxpool,
                tc.tile_pool(name="pxo", bufs=4) as pxopool,
                tc.tile_pool(name="ppre", bufs=2, space="PSUM") as ppre,
            ):
                xsT = pxpool.tile([128, 2, T * BS], b16)
                for k in range(2):
                    for c in range(8):
                        cs = slice(c * 512, (c + 1) * 512)
                        nc.sync.dma_start(xsT[:, k, cs], xsT_d[k, :, cs])
                wxg = pxpool.tile([128, 2, H3], b16)
                for k in range(2):
                    for c in range(6):
                        cs = slice(c * 512, (c + 1) * 512)
                        nc.sync.dma_start(wxg[:, k, cs], wxg_d[k, :, cs])
                biasg = pxpool.tile([1, H3], b16)
                nc.sync.dma_start(biasg[:], biasg_d[:])

                for m in range(32):          # M-tiles: 128 rows = 16 t x 8 b
                    ms = slice(m * 128, (m + 1) * 128)
                    t0 = m * 16
                    for c in range(8):       # N-chunks of 384 (half-groups)
                        j, hh = c // 2, c % 2
                        ncs = slice(j * GF + hh * 384, j * GF + hh * 384 + 384)
                        pp = ppre.tile([128, 384], f32, tag="pp")
                        for k in range(2):
                            nc.tensor.matmul(pp[:], xsT[:, k, ms], wxg[:, k, ncs],
                                             start=(k == 0), stop=False)
                        nc.tensor.matmul(pp[:], ones_b[:],
                                         biasg[:, ncs], start=False, stop=True)
                        ob = pxopool.tile([128, 384], b16, tag="ob")
                        nc.vector.tensor_copy(ob[:], pp[:])
                        nc.sync.dma_start(gx_d[m, :, ncs], ob[:])

            # ---------- phase 2: the scan ----------
            with (
                tc.tile_pool(name="state", bufs=2) as spool,
                tc.tile_pool(name="gxt", bufs=6) as gxpool,
                tc.tile_pool(name="gw", bufs=2) as gwork,
                tc.tile_pool(name="pb0", bufs=2, space="PSUM") as pb0pool,
                tc.tile_pool(name="pbn", bufs=2, space="PSUM") as pbnpool,
                tc.tile_pool(name="ptr", bufs=2, space="PSUM") as ptrpool,
            ):
                hT0 = spool.tile([128, 8, BS], b16, tag="hT")
                nc.gpsimd.memset(hT0[:], 0.0)
                hprev0 = spool.tile([128, Q], f32, tag="hbm")
                nc.gpsimd.memset(hprev0[:], 0.0)

                hT, hprev = hT0, hprev0

                def g(ap, j):
                    return ap[32 * j:32 * j + BS]

                for t in range(t_steps):
                    m, tt = t // 16, t % 16
                    gxt = gxpool.tile([128, GF], b16, tag="gxt")
                    for j in range(4):
                        nc.sync.dma_start(
                            gxt[32 * j:32 * j + BS, :],
                            gx_d[m, tt * BS:(tt + 1) * BS,
                                 j * GF:(j + 1) * GF])

                    pb0 = pb0pool.tile([128, 2 * Q], f32, tag="pb0")
                    pbn = pbnpool.tile([128, Q], f32, tag="pbn")

                    # gate_h: 8 K-rounds x 4 column groups
                    for k in range(8):
                        for j in range(4):
                            nc.tensor.matmul(g(pb0, j), hT[:, k, :],
                                             wh0[:, k, j, :],
                                             start=(k == 0), stop=False,
                                             tile_position=(0, 32 * j))
                        for j in range(4):
                            nc.tensor.matmul(g(pbn, j), hT[:, k, :],
                                             whn[:, k, j, :],
                                             start=(k == 0), stop=False,
                                             tile_position=(0, 32 * j))
                    # inject gate_x (r,i) and bias_hn
                    for j in range(4):
                        nc.tensor.matmul(g(pb0, j), g(identg, j),
                                         gxt[32 * j:32 * j + BS, 0:2 * Q],
                                         start=False, stop=True,
                                         tile_position=(32 * j, 32 * j))
                    for j in range(4):
                        nc.tensor.matmul(g(pbn, j), ones_b[:, 0:BS],
                                         bhn[:, j, :], start=False, stop=True,
                                         tile_position=(0, 32 * j))

                    ri = gwork.tile([128, 2 * Q], f32, tag="ri")
                    ng = gwork.tile([128, Q], f32, tag="ng")
                    t1 = gwork.tile([128, Q], f32, tag="t1")
                    dd = gwork.tile([128, Q], f32, tag="dd")
                    uu = gwork.tile([128, Q], f32, tag="uu")
                    hnew = spool.tile([128, Q], f32, tag="hbm")
                    hTn = spool.tile([128, 8, BS], b16, tag="hT")

                    for j in range(4):
                        # r,i = sigmoid(gh + gx)     [8, 512]
                        nc.scalar.activation(g(ri, j), g(pb0, j), AF.Sigmoid)
                        # t1 = r * (gh_n + bh_n)     [8, 256]
                        nc.vector.tensor_mul(g(t1, j), g(ri, j)[:, 0:Q], g(pbn, j))
                        # t1 += gx_n
                        nc.vector.tensor_add(g(t1, j), g(t1, j),
                                             g(gxt, j)[:, 2 * Q:GF])
                        # ng = tanh(t1)
                        nc.scalar.activation(g(ng, j), g(t1, j), AF.Tanh)
                        # d = h_prev - ng   (gpsimd)
                        nc.gpsimd.tensor_sub(g(dd, j), g(hprev, j), g(ng, j))
                        # u = i * d         (gpsimd)
                        nc.gpsimd.tensor_mul(g(uu, j), g(ri, j)[:, Q:2 * Q], g(dd, j))
                        # h_new = ng + u
                        nc.vector.tensor_add(g(hnew, j), g(ng, j), g(uu, j))
                        # transpose h_new -> hT blocks (2 per group)
                        for hh in range(2):
                            k = 2 * j + hh
                            pt = ptrpool.tile([128, BS], f32, tag="pt")
                            nc.tensor.transpose(
                                pt[:], g(hnew, j)[:, hh * 128:(hh + 1) * 128],
                                g(identfg, j), tile_position=(32 * j, 0))
                            nc.scalar.activation(hTn[:, k, :], pt[:], AF.Copy)

                    if debug and t == 0:
                        nc.sync.dma_start(dbg_gxd_d[:], gx_d[0, 0:BS, :])
                        dtmp = gwork.tile([128, GF], f32, tag="dbg")
                        nc.vector.tensor_copy(dtmp[:], gxt[:])
                        nc.sync.dma_start(dbg_gx_d[:], dtmp[:])
                        nc.sync.dma_start(dbg_ri_d[:], ri[:])
                        dtm2 = gwork.tile([128, Q], f32, tag="dbg2")
                        nc.vector.tensor_copy(dtm2[:], pbn[:])
                        nc.sync.dma_start(dbg_pbn_d[:], dtm2[:])
                        nc.sync.dma_start(dbg_hnew_d[:], hnew[:])
                        dtm3 = gwork.tile([128, 8, BS], f32, tag="dbg3")
                        nc.vector.tensor_copy(dtm3[:], hTn[:])
                        nc.sync.dma_start(dbg_hT_d[:], dtm3[:])

                    hT, hprev = hTn, hnew

                # ---------- final FC ----------
                po = ptrpool.tile([BS, OUT], f32, tag="pt")
                for k in range(8):
                    nc.tensor.matmul(po[:], hT[:, k, :], wfcT[:, k, :],
                                     start=(k == 0), stop=False)
                nc.tensor.matmul(po[:], ones_f[:], bfc[:],
                                 start=False, stop=True)
                ob = gwork.tile([BS, OUT], f32, tag="ri")
                nc.vector.tensor_copy(ob[:], po[:])
                nc.sync.dma_start(out_d[:], ob[:])

    nc.compile()
    return nc


def _prep_inputs(x, Wx, bx, Wh, bh, Wfc, bfc):
    """Host-side layout prep -> list of per-core input dicts."""
    import ml_dtypes
    bf16 = ml_dtypes.bfloat16

    x = np.asarray(x, np.float32)
    Wx = np.asarray(Wx, np.float32)
    bx = np.asarray(bx, np.float32)
    Wh = np.asarray(Wh, np.float32)
    bh = np.asarray(bh, np.float32)
    Wfc = np.asarray(Wfc, np.float32)
    bfc = np.asarray(bfc, np.float32)

    # grouped gate-row order: for quarter j: [r(q_j) | i(q_j) | n(q_j)]
    perm = np.concatenate([
        np.concatenate([np.arange(j * Q, (j + 1) * Q) + g * H for g in range(3)])
        for j in range(4)])                       # [3072] grouped row index

    WxT_g = np.ascontiguousarray(Wx[perm].T)      # [256, 3072-grouped]
    bias_ri = bx + np.concatenate([bh[:2 * H], np.zeros(H, np.float32)])
    biasg = bias_ri[perm].reshape(1, H3)

    WhT = Wh.T                                     # [1024 hid, 3072 gates]
    whb0 = np.empty((8, 128, 4, 2 * Q), np.float32)
    whbn = np.empty((8, 128, 4, Q), np.float32)
    for k in range(8):
        hid = slice(k * 128, (k + 1) * 128)
        for j in range(4):
            q = slice(j * Q, (j + 1) * Q)
            whb0[k, :, j, :Q] = WhT[hid, 0 * H:1 * H][:, q]
            whb0[k, :, j, Q:] = WhT[hid, 1 * H:2 * H][:, q]
            whbn[k, :, j, :] = WhT[hid, 2 * H:3 * H][:, q]
    bhn = bh[2 * H:].reshape(4, Q)[None, :, :]

    wfcT = np.ascontiguousarray(Wfc.T).reshape(8, 128, OUT)
    bfc2 = bfc.reshape(1, OUT)

    common = {
        "wxg": WxT_g.reshape(2, 128, H3).astype(bf16),
        "biasg": biasg.astype(bf16),
        "whb0": whb0.astype(bf16),
        "whbn": whbn.astype(bf16),
        "bhn": bhn.astype(bf16),
        "wfcT": wfcT.astype(bf16),
        "bfc": bfc2,
        "identg": np.tile(np.vstack([np.eye(8, dtype=np.float32),
                                     np.zeros((24, 8), np.float32)]),
                          (4, 1)).astype(bf16),
        "identfg": np.tile(np.vstack([np.eye(8, dtype=np.float32),
                                      np.zeros((24, 8), np.float32)]),
                           (4, 1)),
    }

    in_maps = []
    for c in range(NCORES):
        xs = x[c * BS:(c + 1) * BS]               # [BS, T, IN]
        xsT = xs.transpose(2, 1, 0)               # [IN, T, BS]
        in_maps.append({
            "xsT": np.ascontiguousarray(xsT.reshape(2, 128, T * BS)).astype(bf16),
            **common,
        })
    return in_maps


def kernel(x, Wx, bx, Wh, bh, Wfc, bfc):
    from concourse.bass_utils import run_bass_kernel_spmd

    if "nc" not in _cache:
        _cache["nc"] = _build_program()
    nc = _cache["nc"]

    in_maps = _prep_inputs(x, Wx, bx, Wh, bh, Wfc, bfc)
    res = run_bass_kernel_spmd(nc, in_maps, list(range(NCORES)))
    out = np.concatenate([res.results[c]["out"] for c in range(NCORES)], axis=0)
    return out.astype(np.float32)


if __name__ == "__main__":
    rng = np.random.default_rng(0)
    std = 1.0 / np.sqrt(H)
    inputs = {
        "x": rng.standard_normal((B, T, IN), dtype=np.float32),
        "Wx": rng.uniform(-std, std, (H3, IN)).astype(np.float32),
        "bx": rng.uniform(-std, std, (H3,)).astype(np.float32),
        "Wh": rng.uniform(-std, std, (H3, H)).astype(np.float32),
        "bh": rng.uniform(-std, std, (H3,)).astype(np.float32),
        "Wfc": rng.uniform(-std, std, (OUT, H)).astype(np.float32),
        "bfc": rng.uniform(-std, std, (OUT,)).astype(np.float32),
    }
    out = kernel(**inputs)
    print("out", out.shape, out.dtype)
    print(out[:2])


# revision 18
# speedup vs baseline: 7.1010x; 1.6903x over previous
"""GRU kernel for Trainium2 (8 NeuronCores, SPMD).

Problem: B=64, T=512, IN=256, H=1024, OUT=10
  gate_x_all = x @ Wx.T + bx            [B,T,3H]
  scan over T:  gate_h = h @ Wh.T + bh
                r = sig(i_r + h_r); i = sig(i_i + h_i)
                n = tanh(i_n + r * h_n)
                h = n + i * (h - n)
  out = h_last @ Wfc.T + bfc            [B,OUT]

Strategy (v2): data-parallel over batch (8 rows/core), weights replicated.
 - bf16 operands, fp32 PSUM accumulation and fp32 gate math.
 - Phase 1: gate_x precomputed for all T in one large weight-streaming GEMM
   (output grouped by hidden-quarter, biases folded in via a ones-row).
 - Phase 2: the T-step scan. gate_h computed with 4x column-tiled matmuls
   (hidden quarter j -> PE column group j, PSUM partitions 32j..32j+8),
   so four weight streams flow through the PE concurrently.
   gate_x is re-injected into PSUM via tiny identity-stationary matmuls.
   Gate elementwise work is split across ACT / DVE / GPSIMD.
 - h is kept feature-major (hT) for the next matmul via PE transposes.
"""

import os
import sys

import numpy as np

for p in ("/root/.axon_site", "/root/.axon_site/_ro/trn_rl_repo",
          "/root/.axon_site/_ro/pypackages", "/opt/trn_rl_repo"):
    if p not in sys.path:
        sys.path.append(p)

B, T, IN, H, OUT = 64, 512, 256, 1024, 10
H3 = 3 * H
NCORES = 8
BS = B // NCORES      # batch per core (8)
Q = H // 4            # hidden quarter (256)
GF = 3 * Q            # per-group gate columns (768): [r 256 | i 256 | n 256]

_cache = {}


def _build_program():
    import concourse.bass as bass
    import concourse.bacc as bacc
    import concourse.mybir as mybir
    from concourse.tile import TileContext

    f32 = mybir.dt.float32
    b16 = mybir.dt.bfloat16
    AF = mybir.ActivationFunctionType

    nc = bacc.Bacc(None, target_bir_lowering=False)

    # ---- per-core inputs ----
    # x slice, feature-major for the precompute stationaries: [2,128,(t,b)]
    xsT_d = nc.dram_tensor("xsT", [2, 128, T * BS], b16, kind="ExternalInput")
    # Wx^T with grouped column order (+ biases row): [2,128, 4*768]
    wxg_d = nc.dram_tensor("wxg", [2, 128, H3], b16, kind="ExternalInput")
    biasg_d = nc.dram_tensor("biasg", [1, H3], b16, kind="ExternalInput")
    # Wh^T grouped: bank0 (r,i) and bankn (n) streams
    whb0_d = nc.dram_tensor("whb0", [8, 128, 4, 2 * Q], b16, kind="ExternalInput")
    whbn_d = nc.dram_tensor("whbn", [8, 128, 4, Q], b16, kind="ExternalInput")
    bhn_d = nc.dram_tensor("bhn", [1, 4, Q], b16, kind="ExternalInput")
    wfcT_d = nc.dram_tensor("wfcT", [8, 128, OUT], b16, kind="ExternalInput")
    bfc_d = nc.dram_tensor("bfc", [1, OUT], f32, kind="ExternalInput")
    ident_d = nc.dram_tensor("identg", [128, 8], b16, kind="ExternalInput")
    identf_d = nc.dram_tensor("identfg", [128, 8], f32, kind="ExternalInput")
    out_d = nc.dram_tensor("out", [BS, OUT], f32, kind="ExternalOutput")
    debug = os.environ.get("KERNEL_DEBUG", "0") == "1"
    if debug:
        dbg_gx_d = nc.dram_tensor("dbg_gx", [128, GF], f32, kind="ExternalOutput")
        dbg_ri_d = nc.dram_tensor("dbg_ri", [128, 2 * Q], f32, kind="ExternalOutput")
        dbg_pbn_d = nc.dram_tensor("dbg_pbn", [128, Q], f32, kind="ExternalOutput")
        dbg_hnew_d = nc.dram_tensor("dbg_hnew", [128, Q], f32, kind="ExternalOutput")
        dbg_hT_d = nc.dram_tensor("dbg_hT", [128, 8, BS], f32, kind="ExternalOutput")
        dbg_gxd_d = nc.dram_tensor("dbg_gxd", [BS, H3], b16, kind="ExternalOutput")

    # gate_x for all T, grouped: [m=T/16, p=(16t x 8b), 4*GF]
    gx_d = nc.dram_tensor("gx_all", [T // 16, 128, H3], b16)

    t_steps = int(os.environ.get("KERNEL_T", str(T)))

    with TileContext(nc) as tc:
        # ---------- constants ----------
        with tc.tile_pool(name="const", bufs=1) as cpool:
            wh0 = cpool.tile([128, 8, 4, 2 * Q], b16)
            for k in range(8):
                for j in range(4):
                    nc.sync.dma_start(wh0[:, k, j, :], whb0_d[k, :, j, :])
            whn = cpool.tile([128, 8, 4, Q], b16)
            for k in range(8):
                for j in range(4):
                    nc.sync.dma_start(whn[:, k, j, :], whbn_d[k, :, j, :])
            bhn = cpool.tile([1, 4, Q], b16)
            nc.sync.dma_start(bhn[:], bhn_d[:])
            wfcT = cpool.tile([128, 8, OUT], b16)
            for k in range(8):
                nc.sync.dma_start(wfcT[:, k, :], wfcT_d[k])
            bfc = cpool.tile([1, OUT], f32)
            nc.sync.dma_start(bfc[:], bfc_d[:])
            ones_b = cpool.tile([1, 128], b16)
            nc.gpsimd.memset(ones_b[:], 1.0)
            ones_f = cpool.tile([1, BS], f32)
            nc.gpsimd.memset(ones_f[:], 1.0)
            identg = cpool.tile([128, 8], b16)
            nc.sync.dma_start(identg[:], ident_d[:])
            identfg = cpool.tile([128, 8], f32)
            nc.sync.dma_start(identfg[:], identf_d[:])

            # ---------- phase 1: gate_x precompute ----------
            with (
                tc.tile_pool(name="px", bufs=2) as pxpool,
                tc.tile_pool(name="pxo", bufs=4) as pxopool,
                tc.tile_pool(name="ppre", bufs=2, space="PSUM") as ppre,
            ):
                xsT = pxpool.tile([128, 2, T * BS], b16)
                for k in range(2):
                    for c in range(8):
                        cs = slice(c * 512, (c + 1) * 512)
                        nc.sync.dma_start(xsT[:, k, cs], xsT_d[k, :, cs])
                wxg = pxpool.tile([128, 2, H3], b16)
                for k in range(2):
                    for c in range(6):
                        cs = slice(c * 512, (c + 1) * 512)
                        nc.sync.dma_start(wxg[:, k, cs], wxg_d[k, :, cs])
                biasg = pxpool.tile([1, H3], b16)
                nc.sync.dma_start(biasg[:], biasg_d[:])

                for m in range(32):          # M-tiles: 128 rows = 16 t x 8 b
                    ms = slice(m * 128, (m + 1) * 128)
                    t0 = m * 16
                    for c in range(8):       # N-chunks of 384 (half-groups)
                        j, hh = c // 2, c % 2
                        ncs = slice(j * GF + hh * 384, j * GF + hh * 384 + 384)
                        pp = ppre.tile([128, 384], f32, tag="pp")
                        for k in range(2):
                            nc.tensor.matmul(pp[:], xsT[:, k, ms], wxg[:, k, ncs],
                                             start=(k == 0), stop=False)
                        nc.tensor.matmul(pp[:], ones_b[:],
                                         biasg[:, ncs], start=False, stop=True)
                        ob = pxopool.tile([128, 384], b16, tag="ob")
                        nc.vector.tensor_copy(ob[:], pp[:])
                        nc.sync.dma_start(gx_d[m, :, ncs], ob[:])

            # ---------- phase 2: the scan ----------
            with (
                tc.tile_pool(name="state", bufs=2) as spool,
                tc.tile_pool(name="gxt", bufs=6) as gxpool,
                tc.tile_pool(name="gw", bufs=2) as gwork,
                tc.tile_pool(name="pb0", bufs=2, space="PSUM") as pb0pool,
                tc.tile_pool(name="pbn", bufs=2, space="PSUM") as pbnpool,
                tc.tile_pool(name="ptr", bufs=4, space="PSUM") as ptrpool,
            ):
                hT0 = spool.tile([128, 8, BS], b16, tag="hT")
                nc.gpsimd.memset(hT0[:], 0.0)
                hprev0 = spool.tile([128, Q], f32, tag="hbm")
                nc.gpsimd.memset(hprev0[:], 0.0)

                hT, hprev = hT0, hprev0

                def g(ap, j):
                    return ap[32 * j:32 * j + BS]

                for t in range(t_steps):
                    m, tt = t // 16, t % 16
                    gxt = gxpool.tile([128, GF], b16, tag="gxt")
                    for j in range(4):
                        nc.sync.dma_start(
                            gxt[32 * j:32 * j + BS, :],
                            gx_d[m, tt * BS:(tt + 1) * BS,
                                 j * GF:(j + 1) * GF])

                    pb0 = pb0pool.tile([128, 2 * Q], f32, tag="pb0")
                    pbn = pbnpool.tile([128, Q], f32, tag="pbn")

                    # inject gate_x (r,i) and bias_hn first (no hT dep,
                    # start=True opens the accumulation groups)
                    for j in range(4):
                        nc.tensor.matmul(g(pb0, j), g(identg, j),
                                         gxt[32 * j:32 * j + BS, 0:2 * Q],
                                         start=True, stop=False,
                                         tile_position=(32 * j, 32 * j))
                    for j in range(4):
                        nc.tensor.matmul(g(pbn, j), ones_b[:, 0:BS],
                                         bhn[:, j, :], start=True, stop=False,
                                         tile_position=(0, 32 * j))
                    # gate_h: 8 K-rounds x 4 column groups, k ascending so
                    # each round unblocks as soon as hT block k lands
                    for k in range(8):
                        for j in range(4):
                            nc.tensor.matmul(g(pb0, j), hT[:, k, :],
                                             wh0[:, k, j, :],
                                             start=False, stop=(k == 7),
                                             tile_position=(0, 32 * j))
                        for j in range(4):
                            nc.tensor.matmul(g(pbn, j), hT[:, k, :],
                                             whn[:, k, j, :],
                                             start=False, stop=(k == 7),
                                             tile_position=(0, 32 * j))

                    ri = gwork.tile([128, 2 * Q], f32, tag="ri")
                    ng = gwork.tile([128, Q], f32, tag="ng")
                    t1 = gwork.tile([128, Q], f32, tag="t1")
                    dd = gwork.tile([128, Q], f32, tag="dd")
                    uu = gwork.tile([128, Q], f32, tag="uu")
                    hnew = spool.tile([128, Q], f32, tag="hbm")
                    hTn = spool.tile([128, 8, BS], b16, tag="hT")

                    # single full-partition ops cover all 4 groups at once
                    # (partitions 0..104; rows between groups compute junk)
                    P4 = 3 * 32 + BS
                    nc.scalar.activation(ri[0:P4, :], pb0[0:P4, :], AF.Sigmoid)
                    nc.vector.tensor_mul(t1[0:P4, :], ri[0:P4, 0:Q], pbn[0:P4, :])
                    nc.vector.tensor_add(t1[0:P4, :], t1[0:P4, :],
                                         gxt[0:P4, 2 * Q:GF])
                    nc.scalar.activation(ng[0:P4, :], t1[0:P4, :], AF.Tanh)
                    nc.vector.tensor_sub(dd[0:P4, :], hprev[0:P4, :], ng[0:P4, :])
                    nc.vector.tensor_mul(uu[0:P4, :], ri[0:P4, Q:2 * Q], dd[0:P4, :])
                    nc.vector.tensor_add(hnew[0:P4, :], ng[0:P4, :], uu[0:P4, :])

                    for j in range(4):
                        # transpose h_new -> hT blocks (own bank per group),
                        # then copy so next-step matmuls can start per block
                        pt = ptrpool.tile([128, 2, BS], f32, tag="pt")
                        for hh in range(2):
                            nc.tensor.transpose(
                                pt[:, hh, :], g(hnew, j)[:, hh * 128:(hh + 1) * 128],
                                g(identfg, j), tile_position=(32 * j, 0))
                        nc.vector.tensor_copy(hTn[:, 2 * j:2 * j + 2, :], pt[:])

                    hT, hprev = hTn, hnew

                # ---------- final FC ----------
                po = ptrpool.tile([BS, OUT], f32, tag="pt")
                for k in range(8):
                    nc.tensor.matmul(po[:], hT[:, k, :], wfcT[:, k, :],
                                     start=(k == 0), stop=False)
                nc.tensor.matmul(po[:], ones_f[:], bfc[:],
                                 start=False, stop=True)
                ob = gwork.tile([BS, OUT], f32, tag="ri")
                nc.vector.tensor_copy(ob[:], po[:])
                nc.sync.dma_start(out_d[:], ob[:])

    nc.compile()
    return nc


def _prep_inputs(x, Wx, bx, Wh, bh, Wfc, bfc):
    """Host-side layout prep -> list of per-core input dicts."""
    import ml_dtypes
    bf16 = ml_dtypes.bfloat16

    x = np.asarray(x, np.float32)
    Wx = np.asarray(Wx, np.float32)
    bx = np.asarray(bx, np.float32)
    Wh = np.asarray(Wh, np.float32)
    bh = np.asarray(bh, np.float32)
    Wfc = np.asarray(Wfc, np.float32)
    bfc = np.asarray(bfc, np.float32)

    # grouped gate-row order: for quarter j: [r(q_j) | i(q_j) | n(q_j)]
    perm = np.concatenate([
        np.concatenate([np.arange(j * Q, (j + 1) * Q) + g * H for g in range(3)])
        for j in range(4)])                       # [3072] grouped row index

    WxT_g = np.ascontiguousarray(Wx[perm].T)      # [256, 3072-grouped]
    bias_ri = bx + np.concatenate([bh[:2 * H], np.zeros(H, np.float32)])
    biasg = bias_ri[perm].reshape(1, H3)

    WhT = Wh.T                                     # [1024 hid, 3072 gates]
    whb0 = np.empty((8, 128, 4, 2 * Q), np.float32)
    whbn = np.empty((8, 128, 4, Q), np.float32)
    for k in range(8):
        hid = slice(k * 128, (k + 1) * 128)
        for j in range(4):
            q = slice(j * Q, (j + 1) * Q)
            whb0[k, :, j, :Q] = WhT[hid, 0 * H:1 * H][:, q]
            whb0[k, :, j, Q:] = WhT[hid, 1 * H:2 * H][:, q]
            whbn[k, :, j, :] = WhT[hid, 2 * H:3 * H][:, q]
    bhn = bh[2 * H:].reshape(4, Q)[None, :, :]

    wfcT = np.ascontiguousarray(Wfc.T).reshape(8, 128, OUT)
    bfc2 = bfc.reshape(1, OUT)

    common = {
        "wxg": WxT_g.reshape(2, 128, H3).astype(bf16),
        "biasg": biasg.astype(bf16),
        "whb0": whb0.astype(bf16),
        "whbn": whbn.astype(bf16),
        "bhn": bhn.astype(bf16),
        "wfcT": wfcT.astype(bf16),
        "bfc": bfc2,
        "identg": np.tile(np.vstack([np.eye(8, dtype=np.float32),
                                     np.zeros((24, 8), np.float32)]),
                          (4, 1)).astype(bf16),
        "identfg": np.tile(np.vstack([np.eye(8, dtype=np.float32),
                                      np.zeros((24, 8), np.float32)]),
                           (4, 1)),
    }

    in_maps = []
    for c in range(NCORES):
        xs = x[c * BS:(c + 1) * BS]               # [BS, T, IN]
        xsT = xs.transpose(2, 1, 0)               # [IN, T, BS]
        in_maps.append({
            "xsT": np.ascontiguousarray(xsT.reshape(2, 128, T * BS)).astype(bf16),
            **common,
        })
    return in_maps


def kernel(x, Wx, bx, Wh, bh, Wfc, bfc):
    from concourse.bass_utils import run_bass_kernel_spmd

    if "nc" not in _cache:
        _cache["nc"] = _build_program()
    nc = _cache["nc"]

    in_maps = _prep_inputs(x, Wx, bx, Wh, bh, Wfc, bfc)
    res = run_bass_kernel_spmd(nc, in_maps, list(range(NCORES)))
    out = np.concatenate([res.results[c]["out"] for c in range(NCORES)], axis=0)
    return out.astype(np.float32)


if __name__ == "__main__":
    rng = np.random.default_rng(0)
    std = 1.0 / np.sqrt(H)
    inputs = {
        "x": rng.standard_normal((B, T, IN), dtype=np.float32),
        "Wx": rng.uniform(-std, std, (H3, IN)).astype(np.float32),
        "bx": rng.uniform(-std, std, (H3,)).astype(np.float32),
        "Wh": rng.uniform(-std, std, (H3, H)).astype(np.float32),
        "bh": rng.uniform(-std, std, (H3,)).astype(np.float32),
        "Wfc": rng.uniform(-std, std, (OUT, H)).astype(np.float32),
        "bfc": rng.uniform(-std, std, (OUT,)).astype(np.float32),
    }
    out = kernel(**inputs)
    print("out", out.shape, out.dtype)
    print(out[:2])
